# revision 16
# baseline (speedup 1.0000x reference)
"""Trainium2 Bass kernel: per-class precision/recall sums via fp8 gram matmuls.

Computes, for pred/gt 0-1 indicator tensors of shape [N, C]:
    intersection = sum_n pred*gt   [C]
    pred_sum     = sum_n pred      [C]
    gt_sum       = sum_n gt        [C]
    precisions   = (intersection + EPS) / (pred_sum + EPS)
    recalls      = (intersection + EPS) / (gt_sum + EPS)

Sharding: rows split across 8 NeuronCores. Values are 0/1 indicators, so
the host re-encodes them losslessly as fp8_e4m3 (1 byte/elem) — 4x less
HBM traffic than the f32 originals (16.8 MiB/core, ~40 us at the ~420 GB/s
per-core DMA rate, vs 182 us for f32).

Host staging per core: x[tile=16, p=128, free=8224] fp8, where free is 32
groups of 257 cols: [pred(128) | ones(1) | gt(128)]. A group's 128 cols
are (class c, subrow r) pairs, col = c*8+r; its 128 partitions each hold
a distinct row, so one group covers 1024 rows.

Device pipeline per core:
  - Input DMAs ride the two HWDGE queues (sync + scalar engines) — the
    gpsimd SWDGE path costs ~9 us of descriptor-generation ramp-up.
    Last tile lands in 4 quarters so compute can chase the stream's tail.
    All 16 SBUF slots are resident — no recycling.
  - TensorE, per group: matmul lhsT = pred cols (128-wide), rhs =
    [ones | gt] (N=129), accumulating into ps_gram[128, 129]:
      col 0   = per-(c,r) pred sums   (weights x ones column)
      diag of cols 1..129 = per-(c,r) intersections
  - gt sums are split between the two engines that have slack: VectorE
    strided-reduces groups 0..15 of each tile (~2.3 us/tile) into gtacc;
    TensorE sums groups 16..31 with ones-weight N=512 matmuls (one per
    4-group span) into ps_sum2[1, 512]. The per-tile sum matmuls run
    back-to-back so the ones weights load once per tile and the next
    gram's 128-col weight load hides under their streaming.
  - No device epilogue: DVE copies ps_gram/ps_sum2 to SBUF (DMA cannot
    read PSUM) and the partial tensors go to HBM raw — o1[128, 129],
    o2[1, 512], o3 = gtacc[128, segs*16]. The host extracts the diag,
    folds the span/subrow/partition axes, sums the 8 cores' partials
    (exact integers in f64), and applies the epsilon math.
"""

from contextlib import ExitStack

import numpy as np

N_CORES = 8
N_ROWS, C = 4194304, 16
ROWS_PER_CORE = N_ROWS // N_CORES  # 524288
EPS = np.float32(1e-6)

P = 128              # partitions; also pred/gt cols per group (16 classes x 8 subrows)
R_SUB = 8            # subrows folded into a group's column block
GCOLS = 2 * P + 1    # 257: [pred(128) | ones(1) | gt(128)]
GROUPS_PER_TILE = 64
N_TILES = ROWS_PER_CORE // (P * R_SUB * GROUPS_PER_TILE)  # 16
TILE_FREE = GROUPS_PER_TILE * GCOLS  # 8224
N_QUARTERS = 4       # last tile split so PE/DVE finish right after the stream
SPAN = 4             # groups per TensorE sum-matmul (4 x 128 = 512 = psum bank)

_F8_ONE = np.uint8(0x38)  # 1.0 in float8_e4m3

_CACHE = {}
LAST_RUN = None  # BassKernelResults of the most recent run (for test harness)


def _build_nc(n_tiles=N_TILES, groups_per_tile=GROUPS_PER_TILE):
    import concourse.bass as bass
    import concourse.mybir as mybir

    f32 = mybir.dt.float32
    fp8 = mybir.dt.float8e4

    tile_free = groups_per_tile * GCOLS
    g_half = groups_per_tile // 2          # DVE reduces groups [0, g_half)
    spans = g_half // SPAN                 # TensorE sum-MM spans per tile
    gq = groups_per_tile // N_QUARTERS     # groups per quarter (last tile)
    n_segs = n_tiles + 1                   # gtacc segments (last tile -> 2)
    n_groups = n_tiles * groups_per_tile

    nc = bass.Bass()
    x_d = nc.dram_tensor("x", [n_tiles, P, tile_free], fp8, kind="ExternalInput")
    o1_d = nc.dram_tensor("o1", [P, P + 1], f32, kind="ExternalOutput")
    o2_d = nc.dram_tensor("o2", [1, SPAN * P], f32, kind="ExternalOutput")
    o3_d = nc.dram_tensor("o3", [P, n_segs * C], f32, kind="ExternalOutput")

    ctx = ExitStack()
    with ctx:
        gtacc = ctx.enter_context(nc.sbuf_tensor("gtacc", [P, n_segs * C], f32))
        gbuf = ctx.enter_context(nc.sbuf_tensor("gbuf", [P, P + 1], f32))
        s2buf = ctx.enter_context(nc.sbuf_tensor("s2buf", [1, SPAN * P], f32))
        slots = [
            ctx.enter_context(nc.sbuf_tensor(f"xt{t}", [P, tile_free], fp8))
            for t in range(n_tiles)
        ]

        ps_gram = ctx.enter_context(nc.psum_tensor([P, P + 1], f32))
        ps_sum2 = ctx.enter_context(nc.psum_tensor([1, SPAN * P], f32))

        tsems = [
            ctx.enter_context(nc.semaphore(name=f"t{t}"))
            for t in range(n_tiles - 1)
        ]
        qsems = [
            ctx.enter_context(nc.semaphore(name=f"q{k}"))
            for k in range(N_QUARTERS)
        ]
        pe_sem = ctx.enter_context(nc.semaphore(name="pe"))
        v_sem = ctx.enter_context(nc.semaphore(name="vself"))
        out_sem = ctx.enter_context(nc.semaphore(name="outd"))
        block = ctx.enter_context(nc.Block())

        def grouped(slot):
            return slot[:, :].rearrange("p (f col) -> p f col",
                                        f=groups_per_tile)

        def gt_reduce_view(slot, f0, f1):
            # [p, c, f, r] view of the gt sections of groups [f0, f1)
            v = grouped(slot)[:, f0:f1, P + 1:GCOLS]
            return v.rearrange("p f (c r) -> p c f r", r=R_SUB)

        last = n_tiles - 1
        qf = tile_free // N_QUARTERS

        @block.sync
        def _(sync):
            for t in range(0, n_tiles - 1, 2):
                sync.dma_start(slots[t][:], x_d[t]).then_inc(tsems[t], 16)
            # partial outputs, once DVE finished its reduces + psum copies
            sync.wait_ge(v_sem, n_segs + 2)
            sync.dma_start(o1_d[:, :], gbuf[:]).then_inc(out_sem, 16)
            sync.dma_start(o2_d[:, :], s2buf[:]).then_inc(out_sem, 16)
            sync.dma_start(o3_d[:, :], gtacc[:]).then_inc(out_sem, 16)
            sync.wait_ge(out_sem, 48)

        @block.scalar
        def _(scalar):
            for t in range(1, n_tiles - 1, 2):
                scalar.dma_start(slots[t][:], x_d[t]).then_inc(tsems[t], 16)
            for k in range(N_QUARTERS):
                scalar.dma_start(
                    slots[last][:, k * qf:(k + 1) * qf],
                    x_d[last][:, k * qf:(k + 1) * qf],
                ).then_inc(qsems[k], 16)

        @block.vector
        def _(vector):
            for t in range(n_tiles - 1):
                vector.wait_ge(tsems[t], 16)
                vector.tensor_reduce(
                    gtacc[:, t * C:(t + 1) * C],
                    gt_reduce_view(slots[t], 0, g_half),
                    axis=mybir.AxisListType.XY,
                    op=mybir.AluOpType.add).then_inc(v_sem, 1)
            # last tile: its DVE half arrives as quarters 0 and 1
            for k in range(2):
                vector.wait_ge(qsems[k], 16)
                seg = n_tiles - 1 + k
                vector.tensor_reduce(
                    gtacc[:, seg * C:(seg + 1) * C],
                    gt_reduce_view(slots[last], k * gq, (k + 1) * gq),
                    axis=mybir.AxisListType.XY,
                    op=mybir.AluOpType.add).then_inc(v_sem, 1)
            # copy the psum partials to SBUF so DMA can ship them
            vector.wait_ge(pe_sem, 1)
            vector.tensor_scalar_mul(gbuf[:, :], ps_gram[:, :],
                                     1.0).then_inc(v_sem, 1)
            vector.tensor_scalar_mul(s2buf[:, :], ps_sum2[:, :],
                                     1.0).then_inc(v_sem, 1)

        @block.tensor
        def _(tensor):
            mm = [0, 0]  # gram count, sum count

            def gram(t, g):
                base = g * GCOLS
                inst = nc.tensor.matmul(
                    ps_gram[:, :],
                    slots[t][:, base:base + P],
                    slots[t][:, base + P:base + GCOLS],
                    start=(mm[0] == 0), stop=(mm[0] == n_groups - 1))
                mm[0] += 1
                return inst

            def gtsum(t, j):
                f0 = g_half + j * SPAN
                inst = nc.tensor.matmul(
                    ps_sum2[:, :],
                    slots[t][:, P:P + 1],  # group 0's staged ones column
                    grouped(slots[t])[:, f0:f0 + SPAN, P + 1:GCOLS],
                    start=(mm[1] == 0), stop=(mm[1] == n_tiles * spans - 1))
                mm[1] += 1
                return inst

            for t in range(n_tiles - 1):
                tensor.wait_ge(tsems[t], 16)
                # sum matmuls back-to-back: one ones-LDW per tile, and the
                # first gram's 128-col LDW pulls ahead under their streaming
                for j in range(spans):
                    gtsum(t, j)
                for g in range(groups_per_tile):
                    gram(t, g)
            # last tile: chase the quarter DMAs; sum spans are
            # quarter-aligned (issue each span after its last quarter)
            for k in range(N_QUARTERS):
                tensor.wait_ge(qsems[k], 16)
                for j in range(spans):
                    span_last_g = g_half + j * SPAN + SPAN - 1
                    if k * gq <= span_last_g < (k + 1) * gq:
                        gtsum(last, j)
                for g in range(k * gq, (k + 1) * gq):
                    final = gram(last, g)
            # the final main-loop instruction carries the completion inc
            final.then_inc(pe_sem, 1)
            assert mm[0] == n_groups and mm[1] == n_tiles * spans

    return nc


def _pack_core(pred_c, gt_c, n_tiles=N_TILES, groups_per_tile=GROUPS_PER_TILE):
    """Stage one core's rows as [n_tiles, P, tile_free] fp8 bytes (uint8)."""
    shp = (n_tiles, P, groups_per_tile, R_SUB, C)
    pc = np.asarray(pred_c).reshape(shp)
    gc = np.asarray(gt_c).reshape(shp)
    X = np.empty((n_tiles, P, groups_per_tile, GCOLS), np.uint8)
    # cols are (c, r) pairs, col = c*R_SUB + r -> transpose r and c
    X[..., 0:P] = (pc.transpose(0, 1, 2, 4, 3) != 0).reshape(
        n_tiles, P, groups_per_tile, P) * _F8_ONE
    X[..., P] = _F8_ONE
    X[..., P + 1:GCOLS] = (gc.transpose(0, 1, 2, 4, 3) != 0).reshape(
        n_tiles, P, groups_per_tile, P) * _F8_ONE
    return X.reshape(n_tiles, P, groups_per_tile * GCOLS)


def _unpack_out(o1, o2, o3):
    """Fold one core's raw partials (f64) -> (I, pred_sum, gt_sum), each [C]."""
    diag = o1[np.arange(P), 1 + np.arange(P)]        # I by (c, r)
    inter = diag.reshape(C, R_SUB).sum(axis=1)
    pred_sum = o1[:, 0].reshape(C, R_SUB).sum(axis=1)
    gt_pe = o2.reshape(SPAN, P).sum(axis=0).reshape(C, R_SUB).sum(axis=1)
    gt_dve = o3.reshape(P, -1, C).sum(axis=(0, 1))
    return inter, pred_sum, gt_dve + gt_pe


def _get_nc():
    if "nc" not in _CACHE:
        _CACHE["nc"] = _build_nc()
    return _CACHE["nc"]


def kernel(pred, gt, **run_kwargs):
    global LAST_RUN
    import ml_dtypes
    from concourse.bass_utils import run_bass_kernel_spmd

    pred = np.asarray(pred)
    gt = np.asarray(gt)
    assert pred.shape == (N_ROWS, C) and gt.shape == (N_ROWS, C)

    in_maps = []
    for i in range(N_CORES):
        sl = slice(i * ROWS_PER_CORE, (i + 1) * ROWS_PER_CORE)
        X = _pack_core(pred[sl], gt[sl])
        in_maps.append({"x": X.view(ml_dtypes.float8_e4m3)})

    nc = _get_nc()
    br = run_bass_kernel_spmd(nc, in_maps, core_ids=list(range(N_CORES)),
                              **run_kwargs)
    LAST_RUN = br

    inter = np.zeros(C)
    pred_sum = np.zeros(C)
    gt_sum = np.zeros(C)
    for r in br.results:
        i_, p_, g_ = _unpack_out(r["o1"].astype(np.float64),
                                 r["o2"].astype(np.float64).reshape(-1),
                                 r["o3"].astype(np.float64))
        inter += i_
        pred_sum += p_
        gt_sum += g_
    inter = inter.astype(np.float32)
    pred_sum = pred_sum.astype(np.float32)
    gt_sum = gt_sum.astype(np.float32)

    recalls = (inter + EPS) / (gt_sum + EPS)
    precisions = (inter + EPS) / (pred_sum + EPS)
    return (precisions, recalls, inter, gt_sum, pred_sum)
